# revision 60
# baseline (speedup 1.0000x reference)
"""Trainium2 Bass kernel for the LNN Euler-Lagrange residual.

Math: for a ReLU MLP Lagrangian L(q, qdot) the JAX second-derivative term
d/dt(dL/dqdot) is identically zero (piecewise-linear network), so the
reference output reduces to -dL/dq:

    z1 = x @ W1 + b1          s1 = z1 > 0
    z2 = a1 @ W2 + b2         s2 = z2 > 0      a1 = relu(z1)
    pre1 = s2 @ W2T_eff       (W2T_eff[j,i] = w3[j] * W2[i,j])
    out  = (pre1 * s1) @ (-W1[:32,:].T)

Layout: feature-major (features on partitions, batch as matmul free dim).
Two batch groups are packed on the 128 partitions via host-built 128x128
block stationary matrices, so every matmul uses the full PE array K=128.

The ReLU masks must match an fp32 reference to ~1e-5 of the z scale or
boundary samples flip and corrupt whole 32-wide output rows. Plain fp16
matmuls (~3e-4) flip ~4k samples; so z1 and z2 each use a 3-term fp16
split running at full PE rate (1 cyc/row):

    z = bf16(W)^T xh + fp16(W/64)^T xl + fp16(W - bf16(W))^T xh

with xh = fp16(x), xl = fp16((x - xh)*64) prepared on the host (for L2
the a1 hi/lo split is computed on-device). All correction stationaries
stay in the fp16 normal range, so the residual error is ~3e-6. L3/L4 run
in fp16 on exact {0,1} masks.

Engine budget per pair (1024 cols): PE 16 matmuls (3408ns) is the pacer;
ACT: out-copy + relu + 2 sigmoid-masks (2864); DVE: a1 residual + s1 +
t1 (2646); Pool: a1 fp16 copy (1422). PSUM: z1/p1 pair tiles + 3 z2
chunk buffers + 1 op buffer = 8 banks.
"""

import sys

sys.path.insert(0, "/opt/trn_rl_repo")

from contextlib import ExitStack

import numpy as np

B, D, H = 262144, 32, 64
NCORES = 8
BC = B // NCORES          # samples per core
G = BC // 2               # samples per group (2 groups packed on partitions)
CHUNK = 512               # matmul free-dim (one fp32 PSUM bank)
PAIR = 2 * CHUNK          # elementwise/DMA granularity
NPAIRS = G // PAIR

_CACHE = {}


def _build(bc=BC):
    import concourse.bass as bass
    import concourse.tile as tile
    from concourse import bacc, mybir

    f32 = mybir.dt.float32
    f16 = mybir.dt.float16
    bf16 = mybir.dt.bfloat16
    Relu = mybir.ActivationFunctionType.Relu
    Sigmoid = mybir.ActivationFunctionType.Sigmoid
    is_gt = mybir.AluOpType.is_gt
    mult = mybir.AluOpType.mult
    sub = mybir.AluOpType.subtract
    BIG = 1e18  # sigmoid(BIG*(z2+b2)) saturates to an exact {0,1} bf16 mask

    g = bc // 2
    npairs = g // PAIR

    nc = bacc.Bacc("TRN2", target_bir_lowering=False, debug=False)

    # input planes, rows p = grp*64 + f: xh = fp16(x), xl = fp16((x-xh)*64)
    xH = nc.dram_tensor("xH", [128, g], f16, kind="ExternalInput").ap()
    xL = nc.dram_tensor("xL", [128, g], f16, kind="ExternalInput").ap()
    # stationaries/biases packed in one byte blob -> single prologue DMA:
    # S1h|S1c|S1l|S2h|S2f|S2l|S3 (f16 [128,128] each) | S4 f16 [128,64] |
    # BIASES f32 [128,3]
    CONSTS = nc.dram_tensor("CONSTS", [128, 1932], mybir.dt.uint8,
                            kind="ExternalInput").ap()
    # outT rows (per pair column block): 0:32 A-even, 32:64 B-even,
    # 64:96 A-odd, 96:128 B-odd outputs
    outT = nc.dram_tensor("outT", [128, g // 2], f16,
                          kind="ExternalOutput").ap()

    with tile.TileContext(nc) as tc, ExitStack() as ctx:
        wp = ctx.enter_context(tc.tile_pool(name="w", bufs=1))
        xh_p = ctx.enter_context(tc.tile_pool(name="xh", bufs=3))
        xl_p = ctx.enter_context(tc.tile_pool(name="xl", bufs=3))
        af_p = ctx.enter_context(tc.tile_pool(name="af", bufs=3))
        ah_p = ctx.enter_context(tc.tile_pool(name="ah", bufs=3))
        al_p = ctx.enter_context(tc.tile_pool(name="al", bufs=3))
        s1_p = ctx.enter_context(tc.tile_pool(name="s1", bufs=5))
        s2_p = ctx.enter_context(tc.tile_pool(name="s2", bufs=6))
        t1_p = ctx.enter_context(tc.tile_pool(name="t1", bufs=3))
        ot_p = ctx.enter_context(tc.tile_pool(name="ot", bufs=3))
        # PSUM: z1 2 chunk buffers (short WAR chain through chunk-level
        # relus), p1 single [128,1024] pair tile, 3 z2 chunk buffers (PE
        # lookahead over the mask stage), 1 op buffer.
        z1_p = ctx.enter_context(tc.tile_pool(name="z1", bufs=2, space="PSUM"))
        z2_p = ctx.enter_context(tc.tile_pool(name="z2", bufs=3, space="PSUM"))
        p1_p = ctx.enter_context(tc.tile_pool(name="p1", bufs=1, space="PSUM"))
        op_p = ctx.enter_context(tc.tile_pool(name="op", bufs=1, space="PSUM"))

        cw = wp.tile([128, 1932], mybir.dt.uint8, tag="cw")
        # split the const load so L1(0) only waits for the S1 block
        nc.sync.dma_start(out=cw[:, 0:768], in_=CONSTS[:, 0:768])
        nc.sync.dma_start(out=cw[:, 768:1932], in_=CONSTS[:, 768:1932])
        s1h = cw[:, 0:256].bitcast(f16)
        s1c = cw[:, 256:512].bitcast(f16)
        s1l = cw[:, 512:768].bitcast(f16)
        s2h = cw[:, 768:1024].bitcast(f16)
        s2f = cw[:, 1024:1280].bitcast(f16)
        s2l = cw[:, 1280:1536].bitcast(f16)
        s3w = cw[:, 1536:1792].bitcast(f16)
        s4w = cw[:, 1792:1920].bitcast(f16)
        bia = cw[:, 1920:1932].bitcast(f32)

        # Modulo schedule over pairs, PE-paced: per step s the PE runs
        # L1(s) 6mm, L2(s-2) 6mm, L3(s-3) 2mm, L4(s-4) 2mm. Elementwise:
        # ACT copy(s-5) first (old deps), relu(s), s2(s-2); Pool a1h(s-1);
        # DVE a1l(s-1), s1(s-1), t1(s-3).
        xh_t = {}
        xl_t = {}
        af_t = {}
        ah_t = {}
        al_t = {}
        s1_t = {}
        z1_t = {}
        z2_t = {}
        s2_t = {}
        p1_t = {}
        t1_t = {}
        op_t = {}
        ot_t = {}

        def dma_in(j, split=False):
            xh = xh_p.tile([128, PAIR], f16, tag="xh")
            xl = xl_p.tile([128, PAIR], f16, tag="xl")
            sl = slice(j * PAIR, (j + 1) * PAIR)
            if split:
                for hh in range(2):
                    cs = bass.ts(hh, CHUNK)
                    ds = slice(j * PAIR + hh * CHUNK,
                               j * PAIR + (hh + 1) * CHUNK)
                    nc.sync.dma_start(out=xh[:, cs], in_=xH[:, ds])
                    nc.sync.dma_start(out=xl[:, cs], in_=xL[:, ds])
            else:
                nc.sync.dma_start(out=xh[:], in_=xH[:, sl])
                nc.sync.dma_start(out=xl[:], in_=xL[:, sl])
            xh_t[j] = xh
            xl_t[j] = xl

        for s in range(npairs + 6):
            if s == 0:
                for j in range(min(2, npairs)):
                    dma_in(j, split=(j == 0))

            if 0 <= s - 5 < npairs:
                i = s - 5
                # out-copy of pair i on ACT (inputs are 6 steps old, so it
                # is first in the ACT queue); store on the ACT ring.
                ot = ot_p.tile([128, CHUNK], f16, tag="ot")
                nc.scalar.activation(
                    out=ot[:], in_=op_t[i][:],
                    func=mybir.ActivationFunctionType.Copy)
                nc.scalar.dma_start(
                    out=outT[:, i * CHUNK:(i + 1) * CHUNK], in_=ot[:])
                ot_t[i] = ot
                del op_t[i]

            if s < npairs:
                # L1: z1 = S1h.T xh + S1c.T xl + S1l.T xh (fp16, PSUM acc)
                # then a1f = relu(z1 + b1) -> f32 per chunk on ACT
                a1f = af_p.tile([128, PAIR], f32, tag="af")
                zs = []
                for h in range(2):
                    cs = bass.ts(h, CHUNK)
                    z1p = z1_p.tile([128, CHUNK], f32, tag="z1")
                    nc.tensor.matmul(z1p[:], lhsT=s1h, rhs=xh_t[s][:, cs],
                                     start=True, stop=False)
                    nc.tensor.matmul(z1p[:], lhsT=s1c, rhs=xl_t[s][:, cs],
                                     start=False, stop=False)
                    nc.tensor.matmul(z1p[:], lhsT=s1l, rhs=xh_t[s][:, cs],
                                     start=False, stop=True)
                    nc.scalar.activation(out=a1f[:, cs], in_=z1p[:],
                                         func=Relu,
                                         bias=bia[:, 0:1], scale=1.0)
                    zs.append(z1p)
                z1_t[s] = zs
                af_t[s] = a1f

            if 0 <= s - 1 < npairs:
                i = s - 1
                # a1 hi/lo split + s1 mask (SBUF-side): a1h = fp16(a1f) on
                # Pool; a1l = a1f - a1h (fp16) and s1 = a1h > 0 on DVE.
                a1h = ah_p.tile([128, PAIR], f16, tag="ah")
                nc.gpsimd.tensor_copy(out=a1h[:], in_=af_t[i][:])
                a1l = al_p.tile([128, PAIR], f16, tag="al")
                nc.vector.tensor_tensor(out=a1l[:], in0=af_t[i][:],
                                        in1=a1h[:], op=sub)
                s1m = s1_p.tile([128, PAIR], f16, tag="s1")
                nc.vector.tensor_scalar(out=s1m[:], in0=a1h[:], scalar1=0.0,
                                        scalar2=None, op0=is_gt)
                ah_t[i] = a1h
                al_t[i] = a1l
                s1_t[i] = s1m
                del af_t[i], z1_t[i], xh_t[i], xl_t[i]

            if 0 <= s - 2 < npairs:
                i = s - 2
                # L2: z2 = S2h.T a1h + S2f.T a1l + S2l.T a1h (fp16, acc);
                # s2 = {0,1} mask via saturated sigmoid on ACT per chunk
                zs = []
                ss = []
                for h in range(2):
                    cs = bass.ts(h, CHUNK)
                    z2p = z2_p.tile([128, CHUNK], f32, tag="z2")
                    nc.tensor.matmul(z2p[:], lhsT=s2h, rhs=ah_t[i][:, cs],
                                     start=True, stop=False)
                    nc.tensor.matmul(z2p[:], lhsT=s2f, rhs=al_t[i][:, cs],
                                     start=False, stop=False)
                    nc.tensor.matmul(z2p[:], lhsT=s2l, rhs=ah_t[i][:, cs],
                                     start=False, stop=True)
                    s2m = s2_p.tile([128, CHUNK], f16, tag="s2")
                    nc.scalar.activation(out=s2m[:], in_=z2p[:],
                                         func=Sigmoid,
                                         bias=bia[:, 1:2], scale=BIG)
                    zs.append(z2p)
                    ss.append(s2m)
                z2_t[i] = zs
                s2_t[i] = ss
                del ah_t[i], al_t[i]

            if 0 <= s - 3 < npairs:
                i = s - 3
                # L3: pre1 = S3.T @ s2 (bf16); t1 = pre1 * s1 on DVE (pair)
                p1p = p1_p.tile([128, PAIR], f32, tag="p1")
                for h in range(2):
                    nc.tensor.matmul(
                        p1p[:, bass.ts(h, CHUNK)], lhsT=s3w,
                        rhs=s2_t[i][h][:], start=True, stop=True)
                t1 = t1_p.tile([128, PAIR], f16, tag="t1")
                nc.vector.tensor_tensor(out=t1[:], in0=p1p[:],
                                        in1=s1_t[i][:], op=mult)
                p1_t[i] = p1p
                t1_t[i] = t1
                del s2_t[i], z2_t[i], s1_t[i]

            if 0 <= s - 4 < npairs:
                i = s - 4
                # L4: even chunk -> op[0:64], odd chunk -> op[64:128]
                op_ = op_p.tile([128, CHUNK], f32, tag="op")
                nc.tensor.matmul(
                    op_[0:64, :], lhsT=s4w,
                    rhs=t1_t[i][:, 0:CHUNK], start=True, stop=True)
                nc.tensor.matmul(
                    op_[64:128, :], lhsT=s4w,
                    rhs=t1_t[i][:, CHUNK:PAIR], start=True, stop=True)
                op_t[i] = op_
                del t1_t[i], p1_t[i]

            if s + 2 < npairs:
                dma_in(s + 2)

    nc.compile()
    return nc


def _get_nc(bc=BC):
    if bc not in _CACHE:
        _CACHE[bc] = _build(bc)
    return _CACHE[bc]


def _blockdiag(Wb, swap=False):
    S = np.zeros((128, 128), np.float32)
    if swap:
        S[:64, 64:] = Wb
        S[64:, :64] = Wb
    else:
        S[:64, :64] = Wb
        S[64:, 64:] = Wb
    return S


def _host_prep(W1, b1, W2, b2, W3, b3):
    import ml_dtypes

    def bf(v):
        return v.astype(ml_dtypes.bfloat16).astype(np.float32)

    def fp16(v):
        return v.astype(np.float16).astype(np.float32)

    w3 = np.asarray(W3)[:, 0].astype(np.float32)
    W1 = np.asarray(W1, np.float32)
    W2 = np.asarray(W2, np.float32)
    b1 = np.asarray(b1, np.float32)
    b2 = np.asarray(b2, np.float32)

    # 3-term split stationaries (values all land in fp16 normal range)
    S1h = _blockdiag(bf(W1))
    S1c = _blockdiag(fp16(W1 / 64.0))
    S1l = _blockdiag(fp16(W1 - bf(W1)))
    # L2 swaps group halves (A hidden -> partitions 64:128)
    S2h = _blockdiag(bf(W2), swap=True)
    S2f = _blockdiag(fp16(W2), swap=True)
    S2l = _blockdiag(fp16(W2 - bf(W2)), swap=True)

    S3s = (W2 * w3[None, :]).T  # [j, i] = w3[j] * W2[i, j]
    S3 = np.zeros((128, 128), np.float32)
    S3[64:, :64] = S3s  # A: s2 at p64:128 -> pre1 at p0:64
    S3[:64, 64:] = S3s  # B: s2 at p0:64   -> pre1 at p64:128
    S4s = -(W1[:32, :].T)  # [64, 32]
    S4 = np.zeros((128, 64), np.float32)
    S4[:64, :32] = S4s   # A: t1 p0:64   -> out p0:32
    S4[64:, 32:] = S4s   # B: t1 p64:128 -> out p32:64
    BIASES = np.zeros((128, 3), np.float32)
    BIASES[:, 0] = np.concatenate([b1, b1])
    # sigmoid mask bias: sigmoid(BIG*z2 + BIG*b2) = H(z2 + b2)
    BIASES[:, 1] = np.concatenate([b2, b2]) * 1e18
    blob = np.concatenate([
        S1h.astype(np.float16).view(np.uint8),
        S1c.astype(np.float16).view(np.uint8),
        S1l.astype(np.float16).view(np.uint8),
        S2h.astype(np.float16).view(np.uint8),
        S2f.astype(np.float16).view(np.uint8),
        S2l.astype(np.float16).view(np.uint8),
        S3.astype(np.float16).view(np.uint8),
        S4.astype(np.float16).view(np.uint8),
        BIASES.view(np.uint8),
    ], axis=1)  # [128, 1932]
    return {"CONSTS": np.ascontiguousarray(blob)}


def kernel(inputs, W1, b1, W2, b2, W3, b3):
    from concourse.bass_utils import run_bass_kernel_spmd

    x = np.asarray(inputs, np.float32)
    consts = _host_prep(W1, b1, W2, b2, W3, b3)
    xh = x.astype(np.float16)
    xl = ((x - xh.astype(np.float32)) * 64.0).astype(np.float16)

    in_maps = []
    for k in range(NCORES):
        sl = slice(k * BC, (k + 1) * BC)
        xhc, xlc = xh[sl], xl[sl]          # [BC, 64] fp16
        # rows p = grp*64 + f: group A samples [0,G) then group B [G,2G)
        xHk = np.ascontiguousarray(
            np.concatenate([xhc[:G].T, xhc[G:].T], axis=0))  # [128, G]
        xLk = np.ascontiguousarray(
            np.concatenate([xlc[:G].T, xlc[G:].T], axis=0))
        in_maps.append({"xH": xHk, "xL": xLk, **consts})

    nc = _get_nc()
    res = run_bass_kernel_spmd(nc, in_maps, core_ids=list(range(NCORES)),
                               trace=False)
    outs = []
    for k in range(NCORES):
        oT = np.asarray(res.results[k]["outT"], np.float32)  # [128, G//2]
        # rows: eo*64 + grp*32 + f ; cols: pair*512 + j
        v = oT.reshape(2, 2, 32, NPAIRS, CHUNK)
        # -> [grp, pair, eo, j, f] -> [grp, G, 32]
        w = np.transpose(v, (1, 3, 0, 4, 2)).reshape(2, G, 32)
        outs.append(w[0])
        outs.append(w[1])
    out = np.concatenate(outs, axis=0).astype(np.float32)
    kernel._last_result = res
    return out


# revision 79
# speedup vs baseline: 1.0296x; 1.0296x over previous
"""Trainium2 Bass kernel for the LNN Euler-Lagrange residual.

Math: for a ReLU MLP Lagrangian L(q, qdot) the JAX second-derivative term
d/dt(dL/dqdot) is identically zero (piecewise-linear network), so the
reference output reduces to -dL/dq:

    z1 = x @ W1 + b1          s1 = z1 > 0
    z2 = a1 @ W2 + b2         s2 = z2 > 0      a1 = relu(z1)
    pre1 = s2 @ W2T_eff       (W2T_eff[j,i] = w3[j] * W2[i,j])
    out  = (pre1 * s1) @ (-W1[:32,:].T)

Layout: feature-major (features on partitions, batch as matmul free dim).
Two batch groups are packed on the 128 partitions via host-built 128x128
block stationary matrices, so every matmul uses the full PE array K=128.

The ReLU masks must match an fp32 reference to ~1e-5 of the z scale or
boundary samples flip and corrupt whole 32-wide output rows. Plain fp16
matmuls (~3e-4) flip ~4k samples; so z1 and z2 each use a 3-term fp16
split running at full PE rate (1 cyc/row):

    z = bf16(W)^T xh + fp16(W/64)^T xl + fp16(W - bf16(W))^T xh

with xh = fp16(x), xl = fp16((x - xh)*64) prepared on the host (for L2
the a1 hi/lo split is computed on-device). All correction stationaries
stay in the fp16 normal range, so the residual error is ~3e-6. L3/L4 run
in fp16 on exact {0,1} masks.

Engine budget per pair (1024 cols): PE 16 matmuls (3408ns) is the pacer;
ACT: out-copy + relu + 2 sigmoid-masks (2864); DVE: a1 residual + s1 +
t1 (2646); Pool: a1 fp16 copy (1422). PSUM: z1/p1 pair tiles + 3 z2
chunk buffers + 1 op buffer = 8 banks.
"""

import sys

sys.path.insert(0, "/opt/trn_rl_repo")

from contextlib import ExitStack

import numpy as np

B, D, H = 262144, 32, 64
NCORES = 8
BC = B // NCORES          # samples per core
G = BC // 2               # samples per group (2 groups packed on partitions)
CHUNK = 512               # matmul free-dim (one fp32 PSUM bank)
PAIR = 2 * CHUNK          # elementwise/DMA granularity
NPAIRS = G // PAIR

_CACHE = {}


def _build(bc=BC):
    import concourse.bass as bass
    import concourse.tile as tile
    from concourse import bacc, mybir

    f32 = mybir.dt.float32
    f16 = mybir.dt.float16
    bf16 = mybir.dt.bfloat16
    Relu = mybir.ActivationFunctionType.Relu
    Sigmoid = mybir.ActivationFunctionType.Sigmoid
    is_gt = mybir.AluOpType.is_gt
    mult = mybir.AluOpType.mult
    sub = mybir.AluOpType.subtract
    BIG = 1e18  # sigmoid(BIG*(z2+b2)) saturates to an exact {0,1} bf16 mask

    g = bc // 2
    npairs = g // PAIR

    nc = bacc.Bacc("TRN2", target_bir_lowering=False, debug=False)

    # input planes, rows p = grp*64 + f: xh = fp16(x), xl = fp16((x-xh)*64)
    xH = nc.dram_tensor("xH", [128, g], f16, kind="ExternalInput").ap()
    xL = nc.dram_tensor("xL", [128, g], f16, kind="ExternalInput").ap()
    # stationaries/biases packed in one byte blob -> single prologue DMA:
    # S1h|S1c|S1l|S2h|S2f|S2l|S3 (f16 [128,128] each) | S4 f16 [128,64] |
    # BIASES f32 [128,3]
    CONSTS = nc.dram_tensor("CONSTS", [128, 1932], mybir.dt.uint8,
                            kind="ExternalInput").ap()
    # outT rows (per pair column block): 0:32 A-even, 32:64 B-even,
    # 64:96 A-odd, 96:128 B-odd outputs
    outT = nc.dram_tensor("outT", [128, g // 2], f16,
                          kind="ExternalOutput").ap()

    with tile.TileContext(nc) as tc, ExitStack() as ctx:
        wp = ctx.enter_context(tc.tile_pool(name="w", bufs=1))
        xh_p = ctx.enter_context(tc.tile_pool(name="xh", bufs=3))
        xl_p = ctx.enter_context(tc.tile_pool(name="xl", bufs=3))
        af_p = ctx.enter_context(tc.tile_pool(name="af", bufs=3))
        ah_p = ctx.enter_context(tc.tile_pool(name="ah", bufs=3))
        al_p = ctx.enter_context(tc.tile_pool(name="al", bufs=3))
        s1_p = ctx.enter_context(tc.tile_pool(name="s1", bufs=5))
        s2_p = ctx.enter_context(tc.tile_pool(name="s2", bufs=6))
        t1_p = ctx.enter_context(tc.tile_pool(name="t1", bufs=3))
        ot_p = ctx.enter_context(tc.tile_pool(name="ot", bufs=3))
        # PSUM: z1 2 chunk buffers (short WAR chain through chunk-level
        # relus), p1 single [128,1024] pair tile, 3 z2 chunk buffers (PE
        # lookahead over the mask stage), 1 op buffer.
        z1_p = ctx.enter_context(tc.tile_pool(name="z1", bufs=2, space="PSUM"))
        z2_p = ctx.enter_context(tc.tile_pool(name="z2", bufs=3, space="PSUM"))
        p1_p = ctx.enter_context(tc.tile_pool(name="p1", bufs=1, space="PSUM"))
        op_p = ctx.enter_context(tc.tile_pool(name="op", bufs=1, space="PSUM"))

        cw = wp.tile([128, 1932], mybir.dt.uint8, tag="cw")
        # split the const load so L1(0) only waits for the S1 block
        nc.sync.dma_start(out=cw[:, 0:768], in_=CONSTS[:, 0:768])
        nc.sync.dma_start(out=cw[:, 768:1932], in_=CONSTS[:, 768:1932])
        s1h = cw[:, 0:256].bitcast(f16)
        s1c = cw[:, 256:512].bitcast(f16)
        s1l = cw[:, 512:768].bitcast(f16)
        s2h = cw[:, 768:1024].bitcast(f16)
        s2f = cw[:, 1024:1280].bitcast(f16)
        s2l = cw[:, 1280:1536].bitcast(f16)
        s3w = cw[:, 1536:1792].bitcast(f16)
        s4w = cw[:, 1792:1920].bitcast(f16)
        bia = cw[:, 1920:1932].bitcast(f32)

        # Modulo schedule over pairs, PE-paced: per step s the PE runs
        # L1(s) 6mm, L2(s-2) 6mm, L3(s-3) 2mm, L4(s-4) 2mm. Elementwise:
        # ACT copy(s-5) first (old deps), relu(s), s2(s-2); Pool a1h(s-1);
        # DVE a1l(s-1), s1(s-1), t1(s-3).
        xh_t = {}
        xl_t = {}
        af_t = {}
        ah_t = {}
        al_t = {}
        s1_t = {}
        z1_t = {}
        z2_t = {}
        s2_t = {}
        p1_t = {}
        t1_t = {}
        op_t = {}
        ot_t = {}

        def dma_in(j, split=False):
            # xl loads ride the Pool SWDGE ring (bypasses the serializing
            # HWDGE config device); xh loads stay on the SP ring
            xh = xh_p.tile([128, PAIR], f16, tag="xh")
            xl = xl_p.tile([128, PAIR], f16, tag="xl")
            sl = slice(j * PAIR, (j + 1) * PAIR)
            if split:
                for hh in range(2):
                    cs = bass.ts(hh, CHUNK)
                    ds = slice(j * PAIR + hh * CHUNK,
                               j * PAIR + (hh + 1) * CHUNK)
                    nc.gpsimd.dma_start(out=xh[:, cs], in_=xH[:, ds])
                    nc.sync.dma_start(out=xl[:, cs], in_=xL[:, ds])
            else:
                nc.gpsimd.dma_start(out=xh[:], in_=xH[:, sl])
                nc.sync.dma_start(out=xl[:], in_=xL[:, sl])
            xh_t[j] = xh
            xl_t[j] = xl

        for s in range(npairs + 6):
            if s == 0:
                for j in range(min(2, npairs)):
                    dma_in(j, split=(j == 0))

            if 0 <= s - 5 < npairs:
                i = s - 5
                # out-copy of pair i on ACT (inputs are 6 steps old, so it
                # is first in the ACT queue); store on the ACT ring.
                ot = ot_p.tile([128, CHUNK], f16, tag="ot")
                nc.scalar.activation(
                    out=ot[:], in_=op_t[i][:],
                    func=mybir.ActivationFunctionType.Copy)
                nc.scalar.dma_start(
                    out=outT[:, i * CHUNK:(i + 1) * CHUNK], in_=ot[:])
                ot_t[i] = ot
                del op_t[i]

            if s < npairs:
                # L1: z1 = S1h.T xh + S1c.T xl + S1l.T xh (fp16, PSUM acc)
                # then a1f = relu(z1 + b1) -> f32 per chunk on ACT
                a1f = af_p.tile([128, PAIR], f32, tag="af")
                zs = []
                for h in range(2):
                    cs = bass.ts(h, CHUNK)
                    z1p = z1_p.tile([128, CHUNK], f32, tag="z1")
                    nc.tensor.matmul(z1p[:], lhsT=s1h, rhs=xh_t[s][:, cs],
                                     start=True, stop=False)
                    nc.tensor.matmul(z1p[:], lhsT=s1c, rhs=xl_t[s][:, cs],
                                     start=False, stop=False)
                    nc.tensor.matmul(z1p[:], lhsT=s1l, rhs=xh_t[s][:, cs],
                                     start=False, stop=True)
                    nc.scalar.activation(out=a1f[:, cs], in_=z1p[:],
                                         func=Relu,
                                         bias=bia[:, 0:1], scale=1.0)
                    zs.append(z1p)
                z1_t[s] = zs
                af_t[s] = a1f

            if 0 <= s - 1 < npairs:
                i = s - 1
                # a1 hi/lo split + s1 mask (SBUF-side): a1h = fp16(a1f) on
                # Pool; a1l = a1f - a1h (fp16) and s1 = a1h > 0 on DVE.
                a1h = ah_p.tile([128, PAIR], f16, tag="ah")
                nc.gpsimd.tensor_copy(out=a1h[:], in_=af_t[i][:])
                a1l = al_p.tile([128, PAIR], f16, tag="al")
                nc.vector.tensor_tensor(out=a1l[:], in0=af_t[i][:],
                                        in1=a1h[:], op=sub)
                s1m = s1_p.tile([128, PAIR], f16, tag="s1")
                nc.vector.tensor_scalar(out=s1m[:], in0=a1h[:], scalar1=0.0,
                                        scalar2=None, op0=is_gt)
                ah_t[i] = a1h
                al_t[i] = a1l
                s1_t[i] = s1m
                del af_t[i], z1_t[i], xh_t[i], xl_t[i]

            if 0 <= s - 2 < npairs:
                i = s - 2
                # L2: z2 = S2h.T a1h + S2f.T a1l + S2l.T a1h (fp16, acc);
                # s2 = {0,1} mask via saturated sigmoid on ACT per chunk
                zs = []
                ss = []
                for h in range(2):
                    cs = bass.ts(h, CHUNK)
                    z2p = z2_p.tile([128, CHUNK], f32, tag="z2")
                    nc.tensor.matmul(z2p[:], lhsT=s2h, rhs=ah_t[i][:, cs],
                                     start=True, stop=False)
                    nc.tensor.matmul(z2p[:], lhsT=s2f, rhs=al_t[i][:, cs],
                                     start=False, stop=False)
                    nc.tensor.matmul(z2p[:], lhsT=s2l, rhs=ah_t[i][:, cs],
                                     start=False, stop=True)
                    s2m = s2_p.tile([128, CHUNK], f16, tag="s2")
                    nc.scalar.activation(out=s2m[:], in_=z2p[:],
                                         func=Sigmoid,
                                         bias=bia[:, 1:2], scale=BIG)
                    zs.append(z2p)
                    ss.append(s2m)
                z2_t[i] = zs
                s2_t[i] = ss
                del ah_t[i], al_t[i]

            if 0 <= s - 3 < npairs:
                i = s - 3
                # L3: pre1 = S3.T @ s2 (bf16); t1 = pre1 * s1 on DVE (pair)
                p1p = p1_p.tile([128, PAIR], f32, tag="p1")
                for h in range(2):
                    nc.tensor.matmul(
                        p1p[:, bass.ts(h, CHUNK)], lhsT=s3w,
                        rhs=s2_t[i][h][:], start=True, stop=True)
                t1 = t1_p.tile([128, PAIR], f16, tag="t1")
                nc.vector.tensor_tensor(out=t1[:], in0=p1p[:],
                                        in1=s1_t[i][:], op=mult)
                p1_t[i] = p1p
                t1_t[i] = t1
                del s2_t[i], z2_t[i], s1_t[i]

            if 0 <= s - 4 < npairs:
                i = s - 4
                # L4: even chunk -> op[0:64], odd chunk -> op[64:128]
                op_ = op_p.tile([128, CHUNK], f32, tag="op")
                nc.tensor.matmul(
                    op_[0:64, :], lhsT=s4w,
                    rhs=t1_t[i][:, 0:CHUNK], start=True, stop=True)
                nc.tensor.matmul(
                    op_[64:128, :], lhsT=s4w,
                    rhs=t1_t[i][:, CHUNK:PAIR], start=True, stop=True)
                op_t[i] = op_
                del t1_t[i], p1_t[i]

            if s + 2 < npairs:
                dma_in(s + 2)

    nc.compile()
    return nc


def _get_nc(bc=BC):
    if bc not in _CACHE:
        _CACHE[bc] = _build(bc)
    return _CACHE[bc]


def _blockdiag(Wb, swap=False):
    S = np.zeros((128, 128), np.float32)
    if swap:
        S[:64, 64:] = Wb
        S[64:, :64] = Wb
    else:
        S[:64, :64] = Wb
        S[64:, 64:] = Wb
    return S


def _host_prep(W1, b1, W2, b2, W3, b3):
    import ml_dtypes

    def bf(v):
        return v.astype(ml_dtypes.bfloat16).astype(np.float32)

    def fp16(v):
        return v.astype(np.float16).astype(np.float32)

    w3 = np.asarray(W3)[:, 0].astype(np.float32)
    W1 = np.asarray(W1, np.float32)
    W2 = np.asarray(W2, np.float32)
    b1 = np.asarray(b1, np.float32)
    b2 = np.asarray(b2, np.float32)

    # 3-term split stationaries (values all land in fp16 normal range)
    S1h = _blockdiag(bf(W1))
    S1c = _blockdiag(fp16(W1 / 64.0))
    S1l = _blockdiag(fp16(W1 - bf(W1)))
    # L2 swaps group halves (A hidden -> partitions 64:128)
    S2h = _blockdiag(bf(W2), swap=True)
    S2f = _blockdiag(fp16(W2), swap=True)
    S2l = _blockdiag(fp16(W2 - bf(W2)), swap=True)

    S3s = (W2 * w3[None, :]).T  # [j, i] = w3[j] * W2[i, j]
    S3 = np.zeros((128, 128), np.float32)
    S3[64:, :64] = S3s  # A: s2 at p64:128 -> pre1 at p0:64
    S3[:64, 64:] = S3s  # B: s2 at p0:64   -> pre1 at p64:128
    S4s = -(W1[:32, :].T)  # [64, 32]
    S4 = np.zeros((128, 64), np.float32)
    S4[:64, :32] = S4s   # A: t1 p0:64   -> out p0:32
    S4[64:, 32:] = S4s   # B: t1 p64:128 -> out p32:64
    BIASES = np.zeros((128, 3), np.float32)
    BIASES[:, 0] = np.concatenate([b1, b1])
    # sigmoid mask bias: sigmoid(BIG*z2 + BIG*b2) = H(z2 + b2)
    BIASES[:, 1] = np.concatenate([b2, b2]) * 1e18
    blob = np.concatenate([
        S1h.astype(np.float16).view(np.uint8),
        S1c.astype(np.float16).view(np.uint8),
        S1l.astype(np.float16).view(np.uint8),
        S2h.astype(np.float16).view(np.uint8),
        S2f.astype(np.float16).view(np.uint8),
        S2l.astype(np.float16).view(np.uint8),
        S3.astype(np.float16).view(np.uint8),
        S4.astype(np.float16).view(np.uint8),
        BIASES.view(np.uint8),
    ], axis=1)  # [128, 1932]
    return {"CONSTS": np.ascontiguousarray(blob)}


def kernel(inputs, W1, b1, W2, b2, W3, b3):
    from concourse.bass_utils import run_bass_kernel_spmd

    x = np.asarray(inputs, np.float32)
    consts = _host_prep(W1, b1, W2, b2, W3, b3)
    xh = x.astype(np.float16)
    xl = ((x - xh.astype(np.float32)) * 64.0).astype(np.float16)

    in_maps = []
    for k in range(NCORES):
        sl = slice(k * BC, (k + 1) * BC)
        xhc, xlc = xh[sl], xl[sl]          # [BC, 64] fp16
        # rows p = grp*64 + f: group A samples [0,G) then group B [G,2G)
        xHk = np.ascontiguousarray(
            np.concatenate([xhc[:G].T, xhc[G:].T], axis=0))  # [128, G]
        xLk = np.ascontiguousarray(
            np.concatenate([xlc[:G].T, xlc[G:].T], axis=0))
        in_maps.append({"xH": xHk, "xL": xLk, **consts})

    nc = _get_nc()
    res = run_bass_kernel_spmd(nc, in_maps, core_ids=list(range(NCORES)),
                               trace=False)
    outs = []
    for k in range(NCORES):
        oT = np.asarray(res.results[k]["outT"], np.float32)  # [128, G//2]
        # rows: eo*64 + grp*32 + f ; cols: pair*512 + j
        v = oT.reshape(2, 2, 32, NPAIRS, CHUNK)
        # -> [grp, pair, eo, j, f] -> [grp, G, 32]
        w = np.transpose(v, (1, 3, 0, 4, 2)).reshape(2, G, 32)
        outs.append(w[0])
        outs.append(w[1])
    out = np.concatenate(outs, axis=0).astype(np.float32)
    kernel._last_result = res
    return out
